# revision 23
# baseline (speedup 1.0000x reference)
"""Causal single-head attention (B=8, N=2048, D=H=1024, fp32) on 8 TRN2 cores.

Data-parallel: one batch element per NeuronCore. Mixed fp8/bf16 design tuned
to the measured TRN2 PE behavior (1 moving column per cycle regardless of
dtype; fp8 DoubleRow contracts 256/instruction = 2x bf16 MACs, with weight
loads fully overlapped):

  q^T, k^T = fp8(e4m3) DoubleRow projections from x8/W8 (weights pre-scaled
             by 16 out of e4m3's subnormal range; K bias dropped — it is
             softmax-invariant)
  scores   = fp8 DoubleRow q8.k8 (single chain)
  p        = exp(scores * 2^-13) evicted straight to bf16
  V        = plain bf16 projection (elementwise-accurate; V errors hit the
             output directly through attention-concentrated rows)
  out      = bf16 p @ V / rowsum + bv   (bias applied post-normalization —
             it commutes with the attention average)

Everything stays resident in SBUF (no DRAM spills). Rowsums ride the PE as
1-column matmuls against a ones vector; the softmax division is folded into
the output eviction as a per-partition scale.

ATTN_QK_CHAINS=2 adds an x-residual chain to the Q/K projections (more
accuracy, ~55us slower).
"""

import os
import sys
from contextlib import ExitStack

import numpy as np
import ml_dtypes

# The concourse/bass toolchain comes from the container's python path; fall
# back to the /opt copy when running outside the preconfigured interpreter.
try:
    import concourse.bacc as bacc
except ImportError:  # pragma: no cover
    sys.path.insert(0, "/opt/trn_rl_repo")
    import concourse.bacc as bacc

import concourse.mybir as mybir
from concourse.tile import TileContext
from concourse.bass_utils import run_bass_kernel_spmd

# bass_utils imports antenv.axon_hooks when BASS_TRACE is set; provide a stub
# so tracing degrades gracefully instead of crashing if the module is absent.
try:
    import antenv.axon_hooks  # noqa: F401
except ImportError:  # pragma: no cover
    import types

    _m = types.ModuleType("antenv.axon_hooks")
    _m._hook = None
    _m.set_axon_ntff_profile_hook = lambda h: setattr(_m, "_hook", h)
    _m.get_axon_ntff_profile_hook = lambda: _m._hook
    sys.modules["antenv.axon_hooks"] = _m

# The boot-time NTFF hook install degrades silently when the image's antenv
# lacks axon_hooks; re-attempt it against our stub so BASS_TRACE captures
# HW profiles. Harmless no-op when axon or the .so is absent.
try:  # pragma: no cover
    import antenv.axon_hooks as _ah

    if _ah.get_axon_ntff_profile_hook() is None:
        from trn_agent_boot.trn_boot import _ntff_profile_via_ctypes

        _hook = _ntff_profile_via_ctypes("/opt/axon/libaxon_pjrt.so")
        if _hook is not None:
            _ah.set_axon_ntff_profile_hook(_hook)
except Exception:
    pass

B, N, D, H = 8, 2048, 1024, 1024
P = 128
DP = D // (2 * P)    # 4 fp8 contraction pair-tiles (256 deep each)
DT = D // P          # 8 bf16 contraction tiles
HP = H // (2 * P)    # 4 h pair-tiles for the score contraction
NT = N // P          # 16 sequence tiles of 128
IT = N // 512        # 4 query tiles of 512
WS = 16.0            # weight pre-scale: keeps W out of e4m3 subnormals
EXP_SCALE = 1.0 / (np.sqrt(float(H)) * WS * WS)  # 2^-13

F32 = mybir.dt.float32
F8 = mybir.dt.float8e4
BF16 = mybir.dt.bfloat16
F8NP = ml_dtypes.float8_e4m3
BFNP = ml_dtypes.bfloat16
DR = mybir.MatmulPerfMode.DoubleRow

QK_CHAINS = int(os.environ.get("ATTN_QK_CHAINS", "1"))  # 1 or 2

LAST_RESULT = None  # BassKernelResults of the most recent kernel() call
_CACHE = {}


def build_program(qk_chains: int = QK_CHAINS):
    nc = bacc.Bacc("TRN2", target_bir_lowering=False, debug=False)

    x8d = nc.dram_tensor("x8d", [DP, P, 2, N], F8, kind="ExternalInput")
    if qk_chains >= 2:
        xr8d = nc.dram_tensor("xr8d", [DP, P, 2, N], F8, kind="ExternalInput")
    xbd = nc.dram_tensor("xbd", [DT, P, N], BF16, kind="ExternalInput")
    wq8d = nc.dram_tensor("wq8d", [DP, P, 2, H], F8, kind="ExternalInput")
    wk8d = nc.dram_tensor("wk8d", [DP, P, 2, H], F8, kind="ExternalInput")
    wvbd = nc.dram_tensor("wvbd", [DT, P, H], BF16, kind="ExternalInput")
    bqS = nc.dram_tensor("bqS", [P, 8], F32, kind="ExternalInput")
    bvB = nc.dram_tensor("bvB", [P, H], F32, kind="ExternalInput")
    out = nc.dram_tensor("out", [N, H], F32, kind="ExternalOutput")

    Exp = mybir.ActivationFunctionType.Exp
    Identity = mybir.ActivationFunctionType.Identity
    Copy = mybir.ActivationFunctionType.Copy

    with TileContext(nc) as tc:
        with ExitStack() as top:
            const = top.enter_context(tc.tile_pool(name="const", bufs=1))
            kqv = top.enter_context(tc.tile_pool(name="kqv", bufs=1))
            ps_s = top.enter_context(tc.tile_pool(name="pss", bufs=2, space="PSUM"))
            ps_rs = top.enter_context(tc.tile_pool(name="psrs", bufs=1, space="PSUM"))

            ones_bf = const.tile([P, 1], BF16, tag="ones")
            nc.vector.memset(ones_bf[:], 1.0)
            bq_sb = const.tile([P, 8], F32, tag="bq")
            bv_sb = const.tile([P, H], F32, tag="bv")

            kp = [kqv.tile([P, 2, N], F8, tag=f"kp{i}", name=f"kp{i}") for i in range(HP)]
            qp = [kqv.tile([P, 2, N], F8, tag=f"qp{i}", name=f"qp{i}") for i in range(HP)]
            vt = [kqv.tile([P, H], BF16, tag=f"vt{j}", name=f"vt{j}") for j in range(NT)]

            # ---------------- Phase 1: projections (Q, K, then V) ----------------
            with ExitStack() as p1:
                xpool = p1.enter_context(tc.tile_pool(name="xp", bufs=1))
                wpool = p1.enter_context(tc.tile_pool(name="wp", bufs=1))
                ps1 = p1.enter_context(tc.tile_pool(name="ps1", bufs=5, space="PSUM"))

                xp = [xpool.tile([P, 2, N], F8, tag=f"x{d}", name=f"x{d}") for d in range(DP)]
                if qk_chains >= 2:
                    xrp = [xpool.tile([P, 2, N], F8, tag=f"xr{d}", name=f"xr{d}") for d in range(DP)]
                xb = [xpool.tile([P, N], BF16, tag=f"xb{d}", name=f"xb{d}") for d in range(DT)]
                wq = [wpool.tile([P, 2, H], F8, tag=f"wq{d}", name=f"wq{d}") for d in range(DP)]
                wk = [wpool.tile([P, 2, H], F8, tag=f"wk{d}", name=f"wk{d}") for d in range(DP)]
                wvb = [wpool.tile([P, H], BF16, tag=f"wvb{d}", name=f"wvb{d}") for d in range(DT)]

                def load_x_chunk(tiles, dram, nch):
                    cs = slice(nch * 512, (nch + 1) * 512)
                    for d in range(DP):
                        nc.sync.dma_start(tiles[d][:, :, cs], dram.ap()[d, :, :, cs])

                # DMA waves ordered to unblock the Q projection's first psum
                # group (needs wq + x8 column chunk 0) as early as possible;
                # its d=0 pair (gating the first matmul) goes first, split by
                # partition quarters across DMA queues.
                # wq[0] quarters issue from the Sync queue while xp[0]
                # quarters issue from the (otherwise idle) Scalar queue —
                # DMA issue is ~600ns/instruction and serializes per engine.
                # wq rides the Sync queue, x8 the (otherwise idle) Scalar
                # queue throughout wave 1 — DMA issue is ~600ns/instruction
                # and serializes per issuing engine.
                for quarter in range(4):
                    qsl = slice(quarter * 32, (quarter + 1) * 32)
                    nc.sync.dma_start(wq[0][qsl, :, :], wq8d.ap()[0, qsl, :, :])
                    nc.scalar.dma_start(xp[0][qsl, :, 0:512], x8d.ap()[0, qsl, :, 0:512])
                for d in range(1, DP):
                    nc.sync.dma_start(wq[d][:], wq8d.ap()[d])
                    nc.scalar.dma_start(xp[d][:, :, 0:512], x8d.ap()[d, :, :, 0:512])
                if qk_chains >= 2:
                    load_x_chunk(xrp, xr8d, 0)
                nc.sync.dma_start(bq_sb[:], bqS.ap()[:, :])
                for nch in range(1, 4):
                    load_x_chunk(xp, x8d, nch)
                    if qk_chains >= 2:
                        load_x_chunk(xrp, xr8d, nch)
                for d in range(DP):
                    nc.sync.dma_start(wk[d][:], wk8d.ap()[d])
                for d in range(DT):
                    nc.sync.dma_start(xb[d][:], xbd.ap()[d])
                for d in range(DT):
                    nc.sync.dma_start(wvb[d][:], wvbd.ap()[d])
                nc.sync.dma_start(bv_sb[:], bvB.ap()[:, :])

                def proj_qk(wtiles, evict):
                    srcs = [xp, xrp][:qk_chains] if qk_chains >= 2 else [xp]
                    total = DP * len(srcs)
                    for nch in range(4):
                        cs = slice(nch * 512, (nch + 1) * 512)
                        for hb in range(8):
                            ps = ps1.tile([P, 512], F32, tag="ps")
                            mm = 0
                            for src in srcs:
                                for d in range(DP):
                                    nc.tensor.matmul(
                                        ps[:],
                                        wtiles[d][:, :, hb * P:(hb + 1) * P],
                                        src[d][:, :, cs],
                                        start=(mm == 0),
                                        stop=(mm == total - 1),
                                        perf_mode=DR,
                                    )
                                    mm += 1
                            evict(ps, hb, nch)

                def evict_q(ps, hb, nch):
                    cs = slice(nch * 512, (nch + 1) * 512)
                    nc.scalar.activation(
                        qp[hb >> 1][:, hb & 1, cs], ps[:], Identity, bias=bq_sb[:, hb:hb + 1]
                    )

                def evict_k(ps, hb, nch):
                    cs = slice(nch * 512, (nch + 1) * 512)
                    nc.vector.tensor_copy(kp[hb >> 1][:, hb & 1, cs], ps[:])

                proj_qk(wq, evict_q)
                proj_qk(wk, evict_k)

                # --- V = x @ Wv + bv in bf16, kept resident. Folding bv here
                # is exact: sum(p*(v+bv))/sum(p) == sum(p*v)/sum(p) + bv, and
                # it keeps the output eviction a pure scale+DMA. ---
                for nb in range(NT):
                    ns = slice(nb * P, (nb + 1) * P)
                    for hch in range(2):
                        hs = slice(hch * 512, (hch + 1) * 512)
                        ps = ps1.tile([P, 512], F32, tag="ps")
                        for d in range(DT):
                            nc.tensor.matmul(
                                ps[:],
                                xb[d][:, ns],
                                wvb[d][:, hs],
                                start=(d == 0),
                                stop=(d == DT - 1),
                            )
                        nc.vector.tensor_add(vt[nb][:, hs], ps[:], bv_sb[:, hs])

            # ---------------- Phase 2: attention ----------------
            with ExitStack() as p2:
                pt_pool = p2.enter_context(tc.tile_pool(name="pt", bufs=1))
                sm = p2.enter_context(tc.tile_pool(name="sm", bufs=4))
                op_pool = p2.enter_context(tc.tile_pool(name="op", bufs=2))
                ps_av = p2.enter_context(tc.tile_pool(name="psav", bufs=5, space="PSUM"))

                pt = [pt_pool.tile([P, 512], BF16, tag=f"pt{j}", name=f"pt{j}") for j in range(NT)]

                for t in range(IT):
                    i0 = 512 * t
                    jmax = 4 * t + 3

                    # scores^T [key j, query i] -> exp -> bf16 p, causal mask
                    # on the diagonal tiles. Columns below the diagonal cut c
                    # are never read by this t's AV matmuls.
                    for j in range(jmax + 1):
                        c = max(0, j * P - i0)
                        w = 512 - c
                        ps = ps_s.tile([P, 512], F32, tag="ps")
                        for hp_ in range(HP):
                            nc.tensor.matmul(
                                ps[:, 0:w],
                                kp[hp_][:, :, j * P:(j + 1) * P],
                                qp[hp_][:, :, i0 + c:i0 + 512],
                                start=(hp_ == 0),
                                stop=(hp_ == HP - 1),
                                perf_mode=DR,
                            )
                        nc.scalar.activation(pt[j][:, c:512], ps[:, 0:w], Exp, scale=float(EXP_SCALE))
                        if c > 0 or j * P == i0:
                            # keep exp where key j*P+p <= query i0+c+f', else 0
                            nc.gpsimd.affine_select(
                                out=pt[j][:, c:512],
                                in_=pt[j][:, c:512],
                                compare_op=mybir.AluOpType.is_ge,
                                fill=0.0,
                                base=0,
                                channel_multiplier=-1,
                                pattern=[[1, w]],
                            )

                    # attn @ V, row-sums, normalize + bias on eviction
                    for s_ in range(4):
                        g = 4 * t + s_
                        qs = slice(s_ * P, (s_ + 1) * P)
                        pav = [ps_av.tile([P, 512], F32, tag="pav", name="pav") for _ in range(2)]
                        prs = ps_rs.tile([P, 1], F32, tag="prs")
                        for j in range(g + 1):
                            lhsT = pt[j][:, qs]
                            for hch in range(2):
                                nc.tensor.matmul(
                                    pav[hch][:],
                                    lhsT,
                                    vt[j][:, hch * 512:(hch + 1) * 512],
                                    start=(j == 0),
                                    stop=(j == g),
                                )
                            nc.tensor.matmul(
                                prs[:], lhsT, ones_bf[:], start=(j == 0), stop=(j == g)
                            )
                        recip = sm.tile([P, 1], F32, tag="recip")
                        nc.vector.reciprocal(recip[:], prs[:])
                        ot = op_pool.tile([P, H], F32, tag="ot")
                        for hch in range(2):
                            hs = slice(hch * 512, (hch + 1) * 512)
                            nc.scalar.activation(ot[:, hs], pav[hch][:], Copy, scale=recip[:])
                            nc.sync.dma_start(
                                out.ap()[i0 + s_ * P:i0 + (s_ + 1) * P, hs], ot[:, hs]
                            )

    nc.compile()
    return nc


def _get_program():
    key = QK_CHAINS
    if key not in _CACHE:
        _CACHE[key] = build_program(key)
    return _CACHE[key]


def _pair_layout(mat):
    """[D, M] (already fp8) -> [DP, P, 2, M] pair layout, contiguous."""
    d, m_ = mat.shape
    return np.ascontiguousarray(mat.reshape(DP, 2, P, m_).transpose(0, 2, 1, 3))


def prep_inputs(x, Wq, bq, Wk, bk, Wv, bv, qk_chains: int = None):
    if qk_chains is None:
        qk_chains = QK_CHAINS
    x = np.asarray(x, dtype=np.float32)
    Wq = np.asarray(Wq, dtype=np.float32)
    Wk = np.asarray(Wk, dtype=np.float32)
    Wv = np.asarray(Wv, dtype=np.float32)
    bq = np.asarray(bq, dtype=np.float32)
    bv = np.asarray(bv, dtype=np.float32)

    # weight tiles contract over D: rows of W (no transpose) are the pair dim
    wq_l = _pair_layout((Wq * np.float32(WS)).astype(F8NP))
    wk_l = _pair_layout((Wk * np.float32(WS)).astype(F8NP))
    wvb_l = np.ascontiguousarray(Wv.astype(BFNP).reshape(DT, P, H))

    bqS_h = np.ascontiguousarray((bq * np.float32(WS)).reshape(8, P).T)
    bvB_h = np.ascontiguousarray(np.broadcast_to(bv, (P, H))).astype(np.float32)

    in_maps = []
    for b in range(B):
        xb_ = x[b]
        x8 = xb_.astype(F8NP)
        m = {
            "x8d": _pair_layout(np.ascontiguousarray(x8.T)),
            "xbd": np.ascontiguousarray(xb_.T.astype(BFNP).reshape(DT, P, N)),
            "wq8d": wq_l,
            "wk8d": wk_l,
            "wvbd": wvb_l,
            "bqS": bqS_h,
            "bvB": bvB_h,
        }
        if qk_chains >= 2:
            xr8 = (xb_ - x8.astype(np.float32)).astype(F8NP)
            m["xr8d"] = _pair_layout(np.ascontiguousarray(xr8.T))
        in_maps.append(m)
    return in_maps


def kernel(x, Wq, bq, Wk, bk, Wv, bv):
    global LAST_RESULT
    nc = _get_program()
    in_maps = prep_inputs(x, Wq, bq, Wk, bk, Wv, bv)
    res = run_bass_kernel_spmd(nc, in_maps, core_ids=list(range(B)))
    LAST_RESULT = res
    return np.stack([res.results[b]["out"] for b in range(B)], axis=0)


# revision 24
# speedup vs baseline: 1.1811x; 1.1811x over previous
"""Causal single-head attention (B=8, N=2048, D=H=1024, fp32) on 8 TRN2 cores.

Data-parallel: one batch element per NeuronCore. Mixed fp8/bf16 design tuned
to the measured TRN2 PE behavior (1 moving column per cycle regardless of
dtype; fp8 DoubleRow contracts 256/instruction = 2x bf16 MACs, with weight
loads fully overlapped):

  q^T, k^T = fp8(e4m3) DoubleRow projections from x8/W8 (weights pre-scaled
             by 16 out of e4m3's subnormal range; K bias dropped — it is
             softmax-invariant)
  scores   = fp8 DoubleRow q8.k8 (single chain)
  p        = exp(scores * 2^-13) evicted straight to bf16
  V        = plain bf16 projection (elementwise-accurate; V errors hit the
             output directly through attention-concentrated rows)
  out      = bf16 p @ V / rowsum + bv   (bias applied post-normalization —
             it commutes with the attention average)

Everything stays resident in SBUF (no DRAM spills). Rowsums ride the PE as
1-column matmuls against a ones vector; the softmax division is folded into
the output eviction as a per-partition scale.

ATTN_QK_CHAINS=2 adds an x-residual chain to the Q/K projections (more
accuracy, ~55us slower).
"""

import os
import sys
from contextlib import ExitStack

import numpy as np
import ml_dtypes

# The concourse/bass toolchain comes from the container's python path; fall
# back to the /opt copy when running outside the preconfigured interpreter.
try:
    import concourse.bacc as bacc
except ImportError:  # pragma: no cover
    sys.path.insert(0, "/opt/trn_rl_repo")
    import concourse.bacc as bacc

import concourse.mybir as mybir
from concourse.tile import TileContext
from concourse.bass_utils import run_bass_kernel_spmd

# bass_utils imports antenv.axon_hooks when BASS_TRACE is set; provide a stub
# so tracing degrades gracefully instead of crashing if the module is absent.
try:
    import antenv.axon_hooks  # noqa: F401
except ImportError:  # pragma: no cover
    import types

    _m = types.ModuleType("antenv.axon_hooks")
    _m._hook = None
    _m.set_axon_ntff_profile_hook = lambda h: setattr(_m, "_hook", h)
    _m.get_axon_ntff_profile_hook = lambda: _m._hook
    sys.modules["antenv.axon_hooks"] = _m

# The boot-time NTFF hook install degrades silently when the image's antenv
# lacks axon_hooks; re-attempt it against our stub so BASS_TRACE captures
# HW profiles. Harmless no-op when axon or the .so is absent.
try:  # pragma: no cover
    import antenv.axon_hooks as _ah

    if _ah.get_axon_ntff_profile_hook() is None:
        from trn_agent_boot.trn_boot import _ntff_profile_via_ctypes

        _hook = _ntff_profile_via_ctypes("/opt/axon/libaxon_pjrt.so")
        if _hook is not None:
            _ah.set_axon_ntff_profile_hook(_hook)
except Exception:
    pass

B, N, D, H = 8, 2048, 1024, 1024
P = 128
DP = D // (2 * P)    # 4 fp8 contraction pair-tiles (256 deep each)
DT = D // P          # 8 bf16 contraction tiles
HP = H // (2 * P)    # 4 h pair-tiles for the score contraction
NT = N // P          # 16 sequence tiles of 128
IT = N // 512        # 4 query tiles of 512
WS = 16.0            # weight pre-scale: keeps W out of e4m3 subnormals
EXP_SCALE = 1.0 / (np.sqrt(float(H)) * WS * WS)  # 2^-13

F32 = mybir.dt.float32
F8 = mybir.dt.float8e4
BF16 = mybir.dt.bfloat16
F8NP = ml_dtypes.float8_e4m3
BFNP = ml_dtypes.bfloat16
DR = mybir.MatmulPerfMode.DoubleRow

QK_CHAINS = int(os.environ.get("ATTN_QK_CHAINS", "1"))  # 1 or 2

LAST_RESULT = None  # BassKernelResults of the most recent kernel() call
_CACHE = {}


def build_program(qk_chains: int = QK_CHAINS):
    nc = bacc.Bacc("TRN2", target_bir_lowering=False, debug=False)

    x8d = nc.dram_tensor("x8d", [DP, P, 2, N], F8, kind="ExternalInput")
    if qk_chains >= 2:
        xr8d = nc.dram_tensor("xr8d", [DP, P, 2, N], F8, kind="ExternalInput")
    xbd = nc.dram_tensor("xbd", [DT, P, N], BF16, kind="ExternalInput")
    wq8d = nc.dram_tensor("wq8d", [DP, P, 2, H], F8, kind="ExternalInput")
    wk8d = nc.dram_tensor("wk8d", [DP, P, 2, H], F8, kind="ExternalInput")
    wvbd = nc.dram_tensor("wvbd", [DT, P, H], BF16, kind="ExternalInput")
    bqS = nc.dram_tensor("bqS", [P, 8], F32, kind="ExternalInput")
    bvB = nc.dram_tensor("bvB", [P, H], F32, kind="ExternalInput")
    out = nc.dram_tensor("out", [N, H], F32, kind="ExternalOutput")

    Exp = mybir.ActivationFunctionType.Exp
    Identity = mybir.ActivationFunctionType.Identity
    Copy = mybir.ActivationFunctionType.Copy

    with TileContext(nc) as tc:
        with ExitStack() as top:
            const = top.enter_context(tc.tile_pool(name="const", bufs=1))
            kqv = top.enter_context(tc.tile_pool(name="kqv", bufs=1))
            ps_s = top.enter_context(tc.tile_pool(name="pss", bufs=2, space="PSUM"))
            ps_rs = top.enter_context(tc.tile_pool(name="psrs", bufs=1, space="PSUM"))

            ones_bf = const.tile([P, 1], BF16, tag="ones")
            nc.vector.memset(ones_bf[:], 1.0)
            bq_sb = const.tile([P, 8], F32, tag="bq")
            bv_sb = const.tile([P, H], F32, tag="bv")

            kp = [kqv.tile([P, 2, N], F8, tag=f"kp{i}", name=f"kp{i}") for i in range(HP)]
            qp = [kqv.tile([P, 2, N], F8, tag=f"qp{i}", name=f"qp{i}") for i in range(HP)]
            vt = [kqv.tile([P, H], BF16, tag=f"vt{j}", name=f"vt{j}") for j in range(NT)]

            # ---------------- Phase 1: projections (Q, K, then V) ----------------
            with ExitStack() as p1:
                xpool = p1.enter_context(tc.tile_pool(name="xp", bufs=1))
                wpool = p1.enter_context(tc.tile_pool(name="wp", bufs=1))
                ps1 = p1.enter_context(tc.tile_pool(name="ps1", bufs=5, space="PSUM"))

                xp = [xpool.tile([P, 2, N], F8, tag=f"x{d}", name=f"x{d}") for d in range(DP)]
                if qk_chains >= 2:
                    xrp = [xpool.tile([P, 2, N], F8, tag=f"xr{d}", name=f"xr{d}") for d in range(DP)]
                xb = [xpool.tile([P, N], BF16, tag=f"xb{d}", name=f"xb{d}") for d in range(DT)]
                wq = [wpool.tile([P, 2, H], F8, tag=f"wq{d}", name=f"wq{d}") for d in range(DP)]
                wk = [wpool.tile([P, 2, H], F8, tag=f"wk{d}", name=f"wk{d}") for d in range(DP)]
                wvb = [wpool.tile([P, H], BF16, tag=f"wvb{d}", name=f"wvb{d}") for d in range(DT)]

                def load_x_chunk(tiles, dram, nch):
                    cs = slice(nch * 512, (nch + 1) * 512)
                    for d in range(DP):
                        nc.sync.dma_start(tiles[d][:, :, cs], dram.ap()[d, :, :, cs])

                # DMA waves ordered to unblock the Q projection's first psum
                # group (needs wq + x8 column chunk 0) as early as possible;
                # its d=0 pair (gating the first matmul) goes first, split by
                # partition quarters across DMA queues.
                # wq[0] quarters issue from the Sync queue while xp[0]
                # quarters issue from the (otherwise idle) Scalar queue —
                # DMA issue is ~600ns/instruction and serializes per engine.
                # wq rides the Sync queue, x8 the (otherwise idle) Scalar
                # queue throughout wave 1 — DMA issue is ~600ns/instruction
                # and serializes per issuing engine.
                for quarter in range(4):
                    qsl = slice(quarter * 32, (quarter + 1) * 32)
                    nc.sync.dma_start(wq[0][qsl, :, :], wq8d.ap()[0, qsl, :, :])
                    nc.scalar.dma_start(xp[0][qsl, :, 0:512], x8d.ap()[0, qsl, :, 0:512])
                for d in range(1, DP):
                    nc.sync.dma_start(wq[d][:], wq8d.ap()[d])
                    nc.scalar.dma_start(xp[d][:, :, 0:512], x8d.ap()[d, :, :, 0:512])
                if qk_chains >= 2:
                    load_x_chunk(xrp, xr8d, 0)
                nc.sync.dma_start(bq_sb[:], bqS.ap()[:, :])
                for nch in range(1, 4):
                    load_x_chunk(xp, x8d, nch)
                    if qk_chains >= 2:
                        load_x_chunk(xrp, xr8d, nch)
                for d in range(DP):
                    nc.sync.dma_start(wk[d][:], wk8d.ap()[d])
                for d in range(DT):
                    nc.sync.dma_start(xb[d][:], xbd.ap()[d])
                for d in range(DT):
                    nc.sync.dma_start(wvb[d][:], wvbd.ap()[d])
                nc.sync.dma_start(bv_sb[:], bvB.ap()[:, :])

                def proj_qk(wtiles, evict):
                    srcs = [xp, xrp][:qk_chains] if qk_chains >= 2 else [xp]
                    total = DP * len(srcs)
                    for nch in range(4):
                        cs = slice(nch * 512, (nch + 1) * 512)
                        for hb in range(8):
                            ps = ps1.tile([P, 512], F32, tag="ps")
                            mm = 0
                            for src in srcs:
                                for d in range(DP):
                                    nc.tensor.matmul(
                                        ps[:],
                                        wtiles[d][:, :, hb * P:(hb + 1) * P],
                                        src[d][:, :, cs],
                                        start=(mm == 0),
                                        stop=(mm == total - 1),
                                        perf_mode=DR,
                                    )
                                    mm += 1
                            evict(ps, hb, nch)

                def evict_q(ps, hb, nch):
                    cs = slice(nch * 512, (nch + 1) * 512)
                    nc.scalar.activation(
                        qp[hb >> 1][:, hb & 1, cs], ps[:], Identity, bias=bq_sb[:, hb:hb + 1]
                    )

                def evict_k(ps, hb, nch):
                    cs = slice(nch * 512, (nch + 1) * 512)
                    nc.vector.tensor_copy(kp[hb >> 1][:, hb & 1, cs], ps[:])

                proj_qk(wq, evict_q)
                proj_qk(wk, evict_k)

                # --- V = x @ Wv + bv in bf16, kept resident. Folding bv here
                # is exact: sum(p*(v+bv))/sum(p) == sum(p*v)/sum(p) + bv, and
                # it keeps the output eviction a pure scale+DMA. ---
                for nb in range(NT):
                    ns = slice(nb * P, (nb + 1) * P)
                    for hch in range(2):
                        hs = slice(hch * 512, (hch + 1) * 512)
                        ps = ps1.tile([P, 512], F32, tag="ps")
                        for d in range(DT):
                            nc.tensor.matmul(
                                ps[:],
                                xb[d][:, ns],
                                wvb[d][:, hs],
                                start=(d == 0),
                                stop=(d == DT - 1),
                            )
                        nc.vector.tensor_add(vt[nb][:, hs], ps[:], bv_sb[:, hs])

            # ---------------- Phase 2: attention ----------------
            with ExitStack() as p2:
                pt_pool = p2.enter_context(tc.tile_pool(name="pt", bufs=1))
                sm = p2.enter_context(tc.tile_pool(name="sm", bufs=4))
                op_pool = p2.enter_context(tc.tile_pool(name="op", bufs=2))
                ps_av = p2.enter_context(tc.tile_pool(name="psav", bufs=4, space="PSUM"))

                pt = [pt_pool.tile([P, 512], BF16, tag=f"pt{j}", name=f"pt{j}") for j in range(NT)]

                for t in range(IT):
                    i0 = 512 * t
                    jmax = 4 * t + 3

                    # scores^T [key j, query i] -> exp -> bf16 p, causal mask
                    # on the diagonal tiles. Columns below the diagonal cut c
                    # are never read by this t's AV matmuls.
                    for j in range(jmax + 1):
                        c = max(0, j * P - i0)
                        w = 512 - c
                        ps = ps_s.tile([P, 512], F32, tag="ps")
                        for hp_ in range(HP):
                            nc.tensor.matmul(
                                ps[:, 0:w],
                                kp[hp_][:, :, j * P:(j + 1) * P],
                                qp[hp_][:, :, i0 + c:i0 + 512],
                                start=(hp_ == 0),
                                stop=(hp_ == HP - 1),
                                perf_mode=DR,
                            )
                        nc.scalar.activation(pt[j][:, c:512], ps[:, 0:w], Exp, scale=float(EXP_SCALE))
                        if c > 0 or j * P == i0:
                            # keep exp where key j*P+p <= query i0+c+f', else 0
                            nc.gpsimd.affine_select(
                                out=pt[j][:, c:512],
                                in_=pt[j][:, c:512],
                                compare_op=mybir.AluOpType.is_ge,
                                fill=0.0,
                                base=0,
                                channel_multiplier=-1,
                                pattern=[[1, w]],
                            )

                    # attn @ V, row-sums, normalize + bias on eviction
                    for s_ in range(4):
                        g = 4 * t + s_
                        qs = slice(s_ * P, (s_ + 1) * P)
                        pav = [ps_av.tile([P, 512], F32, tag="pav", name="pav") for _ in range(2)]
                        prs = ps_rs.tile([P, 1], F32, tag="prs")
                        for j in range(g + 1):
                            lhsT = pt[j][:, qs]
                            for hch in range(2):
                                nc.tensor.matmul(
                                    pav[hch][:],
                                    lhsT,
                                    vt[j][:, hch * 512:(hch + 1) * 512],
                                    start=(j == 0),
                                    stop=(j == g),
                                )
                            nc.tensor.matmul(
                                prs[:], lhsT, ones_bf[:], start=(j == 0), stop=(j == g)
                            )
                        recip = sm.tile([P, 1], F32, tag="recip")
                        nc.vector.reciprocal(recip[:], prs[:])
                        ot = op_pool.tile([P, H], F32, tag="ot")
                        for hch in range(2):
                            hs = slice(hch * 512, (hch + 1) * 512)
                            nc.scalar.activation(ot[:, hs], pav[hch][:], Copy, scale=recip[:])
                            nc.sync.dma_start(
                                out.ap()[i0 + s_ * P:i0 + (s_ + 1) * P, hs], ot[:, hs]
                            )

    nc.compile()
    return nc


def _get_program():
    key = QK_CHAINS
    if key not in _CACHE:
        _CACHE[key] = build_program(key)
    return _CACHE[key]


def _pair_layout(mat):
    """[D, M] (already fp8) -> [DP, P, 2, M] pair layout, contiguous."""
    d, m_ = mat.shape
    return np.ascontiguousarray(mat.reshape(DP, 2, P, m_).transpose(0, 2, 1, 3))


def prep_inputs(x, Wq, bq, Wk, bk, Wv, bv, qk_chains: int = None):
    if qk_chains is None:
        qk_chains = QK_CHAINS
    x = np.asarray(x, dtype=np.float32)
    Wq = np.asarray(Wq, dtype=np.float32)
    Wk = np.asarray(Wk, dtype=np.float32)
    Wv = np.asarray(Wv, dtype=np.float32)
    bq = np.asarray(bq, dtype=np.float32)
    bv = np.asarray(bv, dtype=np.float32)

    # weight tiles contract over D: rows of W (no transpose) are the pair dim
    wq_l = _pair_layout((Wq * np.float32(WS)).astype(F8NP))
    wk_l = _pair_layout((Wk * np.float32(WS)).astype(F8NP))
    wvb_l = np.ascontiguousarray(Wv.astype(BFNP).reshape(DT, P, H))

    bqS_h = np.ascontiguousarray((bq * np.float32(WS)).reshape(8, P).T)
    bvB_h = np.ascontiguousarray(np.broadcast_to(bv, (P, H))).astype(np.float32)

    in_maps = []
    for b in range(B):
        xb_ = x[b]
        x8 = xb_.astype(F8NP)
        m = {
            "x8d": _pair_layout(np.ascontiguousarray(x8.T)),
            "xbd": np.ascontiguousarray(xb_.T.astype(BFNP).reshape(DT, P, N)),
            "wq8d": wq_l,
            "wk8d": wk_l,
            "wvbd": wvb_l,
            "bqS": bqS_h,
            "bvB": bvB_h,
        }
        if qk_chains >= 2:
            xr8 = (xb_ - x8.astype(np.float32)).astype(F8NP)
            m["xr8d"] = _pair_layout(np.ascontiguousarray(xr8.T))
        in_maps.append(m)
    return in_maps


def kernel(x, Wq, bq, Wk, bk, Wv, bv):
    global LAST_RESULT
    nc = _get_program()
    in_maps = prep_inputs(x, Wq, bq, Wk, bk, Wv, bv)
    res = run_bass_kernel_spmd(nc, in_maps, core_ids=list(range(B)))
    LAST_RESULT = res
    return np.stack([res.results[b]["out"] for b in range(B)], axis=0)


# revision 25
# speedup vs baseline: 1.2023x; 1.0180x over previous
"""Causal single-head attention (B=8, N=2048, D=H=1024, fp32) on 8 TRN2 cores.

Data-parallel: one batch element per NeuronCore. Mixed fp8/bf16 design tuned
to the measured TRN2 PE behavior (1 moving column per cycle regardless of
dtype; fp8 DoubleRow contracts 256/instruction = 2x bf16 MACs, with weight
loads fully overlapped):

  q^T, k^T = fp8(e4m3) DoubleRow projections from x8/W8 (weights pre-scaled
             by 16 out of e4m3's subnormal range; K bias dropped — it is
             softmax-invariant)
  scores   = fp8 DoubleRow q8.k8 (single chain)
  p        = exp(scores * 2^-13) evicted straight to bf16
  V        = plain bf16 projection (elementwise-accurate; V errors hit the
             output directly through attention-concentrated rows)
  out      = bf16 p @ V / rowsum + bv   (bias applied post-normalization —
             it commutes with the attention average)

Everything stays resident in SBUF (no DRAM spills). Rowsums ride the PE as
1-column matmuls against a ones vector; the softmax division is folded into
the output eviction as a per-partition scale.

ATTN_QK_CHAINS=2 adds an x-residual chain to the Q/K projections (more
accuracy, ~55us slower).
"""

import os
import sys
from contextlib import ExitStack

import numpy as np
import ml_dtypes

# The concourse/bass toolchain comes from the container's python path; fall
# back to the /opt copy when running outside the preconfigured interpreter.
try:
    import concourse.bacc as bacc
except ImportError:  # pragma: no cover
    sys.path.insert(0, "/opt/trn_rl_repo")
    import concourse.bacc as bacc

import concourse.mybir as mybir
from concourse.tile import TileContext
from concourse.bass_utils import run_bass_kernel_spmd

# bass_utils imports antenv.axon_hooks when BASS_TRACE is set; provide a stub
# so tracing degrades gracefully instead of crashing if the module is absent.
try:
    import antenv.axon_hooks  # noqa: F401
except ImportError:  # pragma: no cover
    import types

    _m = types.ModuleType("antenv.axon_hooks")
    _m._hook = None
    _m.set_axon_ntff_profile_hook = lambda h: setattr(_m, "_hook", h)
    _m.get_axon_ntff_profile_hook = lambda: _m._hook
    sys.modules["antenv.axon_hooks"] = _m

# The boot-time NTFF hook install degrades silently when the image's antenv
# lacks axon_hooks; re-attempt it against our stub so BASS_TRACE captures
# HW profiles. Harmless no-op when axon or the .so is absent.
try:  # pragma: no cover
    import antenv.axon_hooks as _ah

    if _ah.get_axon_ntff_profile_hook() is None:
        from trn_agent_boot.trn_boot import _ntff_profile_via_ctypes

        _hook = _ntff_profile_via_ctypes("/opt/axon/libaxon_pjrt.so")
        if _hook is not None:
            _ah.set_axon_ntff_profile_hook(_hook)
except Exception:
    pass

B, N, D, H = 8, 2048, 1024, 1024
P = 128
DP = D // (2 * P)    # 4 fp8 contraction pair-tiles (256 deep each)
DT = D // P          # 8 bf16 contraction tiles
HP = H // (2 * P)    # 4 h pair-tiles for the score contraction
NT = N // P          # 16 sequence tiles of 128
IT = N // 512        # 4 query tiles of 512
WS = 16.0            # weight pre-scale: keeps W out of e4m3 subnormals
EXP_SCALE = 1.0 / (np.sqrt(float(H)) * WS * WS)  # 2^-13

F32 = mybir.dt.float32
F8 = mybir.dt.float8e4
BF16 = mybir.dt.bfloat16
F8NP = ml_dtypes.float8_e4m3
BFNP = ml_dtypes.bfloat16
DR = mybir.MatmulPerfMode.DoubleRow

QK_CHAINS = int(os.environ.get("ATTN_QK_CHAINS", "1"))  # 1 or 2

LAST_RESULT = None  # BassKernelResults of the most recent kernel() call
_CACHE = {}


def build_program(qk_chains: int = QK_CHAINS):
    nc = bacc.Bacc("TRN2", target_bir_lowering=False, debug=False)

    x8d = nc.dram_tensor("x8d", [DP, P, 2, N], F8, kind="ExternalInput")
    if qk_chains >= 2:
        xr8d = nc.dram_tensor("xr8d", [DP, P, 2, N], F8, kind="ExternalInput")
    xbd = nc.dram_tensor("xbd", [DT, P, N], BF16, kind="ExternalInput")
    wq8d = nc.dram_tensor("wq8d", [DP, P, 2, H], F8, kind="ExternalInput")
    wk8d = nc.dram_tensor("wk8d", [DP, P, 2, H], F8, kind="ExternalInput")
    wvbd = nc.dram_tensor("wvbd", [DT, P, H], BF16, kind="ExternalInput")
    bqS = nc.dram_tensor("bqS", [P, 8], F32, kind="ExternalInput")
    bvB = nc.dram_tensor("bvB", [P, H], F32, kind="ExternalInput")
    out = nc.dram_tensor("out", [N, H], F32, kind="ExternalOutput")

    Exp = mybir.ActivationFunctionType.Exp
    Identity = mybir.ActivationFunctionType.Identity
    Copy = mybir.ActivationFunctionType.Copy

    with TileContext(nc) as tc:
        with ExitStack() as top:
            const = top.enter_context(tc.tile_pool(name="const", bufs=1))
            kqv = top.enter_context(tc.tile_pool(name="kqv", bufs=1))
            ps_s = top.enter_context(tc.tile_pool(name="pss", bufs=2, space="PSUM"))
            ps_rs = top.enter_context(tc.tile_pool(name="psrs", bufs=1, space="PSUM"))

            ones_bf = const.tile([P, 1], BF16, tag="ones")
            nc.vector.memset(ones_bf[:], 1.0)
            bq_sb = const.tile([P, 8], F32, tag="bq")
            bv_sb = const.tile([P, H], F32, tag="bv")

            kp = [kqv.tile([P, 2, N], F8, tag=f"kp{i}", name=f"kp{i}") for i in range(HP)]
            qp = [kqv.tile([P, 2, N], F8, tag=f"qp{i}", name=f"qp{i}") for i in range(HP)]
            vt = [kqv.tile([P, H], BF16, tag=f"vt{j}", name=f"vt{j}") for j in range(NT)]

            # ---------------- Phase 1: projections (Q, K, then V) ----------------
            with ExitStack() as p1:
                xpool = p1.enter_context(tc.tile_pool(name="xp", bufs=1))
                wpool = p1.enter_context(tc.tile_pool(name="wp", bufs=1))
                ps1 = p1.enter_context(tc.tile_pool(name="ps1", bufs=5, space="PSUM"))

                xp = [xpool.tile([P, 2, N], F8, tag=f"x{d}", name=f"x{d}") for d in range(DP)]
                if qk_chains >= 2:
                    xrp = [xpool.tile([P, 2, N], F8, tag=f"xr{d}", name=f"xr{d}") for d in range(DP)]
                xb = [xpool.tile([P, N], BF16, tag=f"xb{d}", name=f"xb{d}") for d in range(DT)]
                wq = [wpool.tile([P, 2, H], F8, tag=f"wq{d}", name=f"wq{d}") for d in range(DP)]
                wk = [wpool.tile([P, 2, H], F8, tag=f"wk{d}", name=f"wk{d}") for d in range(DP)]
                wvb = [wpool.tile([P, H], BF16, tag=f"wvb{d}", name=f"wvb{d}") for d in range(DT)]

                def load_x_chunk(tiles, dram, nch):
                    cs = slice(nch * 512, (nch + 1) * 512)
                    for d in range(DP):
                        nc.sync.dma_start(tiles[d][:, :, cs], dram.ap()[d, :, :, cs])

                # DMA waves ordered to unblock the Q projection's first psum
                # group (needs wq + x8 column chunk 0) as early as possible;
                # its d=0 pair (gating the first matmul) goes first, split by
                # partition quarters across DMA queues.
                # wq[0] quarters issue from the Sync queue while xp[0]
                # quarters issue from the (otherwise idle) Scalar queue —
                # DMA issue is ~600ns/instruction and serializes per engine.
                # wq rides the Sync queue, x8 the (otherwise idle) Scalar
                # queue throughout wave 1 — DMA issue is ~600ns/instruction
                # and serializes per issuing engine.
                for quarter in range(4):
                    qsl = slice(quarter * 32, (quarter + 1) * 32)
                    nc.sync.dma_start(wq[0][qsl, :, :], wq8d.ap()[0, qsl, :, :])
                    nc.scalar.dma_start(xp[0][qsl, :, 0:512], x8d.ap()[0, qsl, :, 0:512])
                for d in range(1, DP):
                    nc.sync.dma_start(wq[d][:], wq8d.ap()[d])
                    nc.scalar.dma_start(xp[d][:, :, 0:512], x8d.ap()[d, :, :, 0:512])
                if qk_chains >= 2:
                    load_x_chunk(xrp, xr8d, 0)
                nc.sync.dma_start(bq_sb[:], bqS.ap()[:, :])
                for nch in range(1, 4):
                    load_x_chunk(xp, x8d, nch)
                    if qk_chains >= 2:
                        load_x_chunk(xrp, xr8d, nch)
                for d in range(DP):
                    nc.sync.dma_start(wk[d][:], wk8d.ap()[d])
                for d in range(DT):
                    nc.sync.dma_start(xb[d][:], xbd.ap()[d])
                for d in range(DT):
                    nc.sync.dma_start(wvb[d][:], wvbd.ap()[d])
                nc.sync.dma_start(bv_sb[:], bvB.ap()[:, :])

                def proj_qk(wtiles, evict):
                    srcs = [xp, xrp][:qk_chains] if qk_chains >= 2 else [xp]
                    total = DP * len(srcs)
                    for nch in range(4):
                        cs = slice(nch * 512, (nch + 1) * 512)
                        for hb in range(8):
                            ps = ps1.tile([P, 512], F32, tag="ps")
                            mm = 0
                            for src in srcs:
                                for d in range(DP):
                                    nc.tensor.matmul(
                                        ps[:],
                                        wtiles[d][:, :, hb * P:(hb + 1) * P],
                                        src[d][:, :, cs],
                                        start=(mm == 0),
                                        stop=(mm == total - 1),
                                        perf_mode=DR,
                                    )
                                    mm += 1
                            evict(ps, hb, nch)

                def evict_q(ps, hb, nch):
                    cs = slice(nch * 512, (nch + 1) * 512)
                    nc.scalar.activation(
                        qp[hb >> 1][:, hb & 1, cs], ps[:], Identity, bias=bq_sb[:, hb:hb + 1]
                    )

                def evict_k(ps, hb, nch):
                    cs = slice(nch * 512, (nch + 1) * 512)
                    nc.vector.tensor_copy(kp[hb >> 1][:, hb & 1, cs], ps[:])

                proj_qk(wq, evict_q)
                proj_qk(wk, evict_k)

                # --- V = x @ Wv + bv in bf16, kept resident. Folding bv here
                # is exact: sum(p*(v+bv))/sum(p) == sum(p*v)/sum(p) + bv, and
                # it keeps the output eviction a pure scale+DMA. ---
                for nb in range(NT):
                    ns = slice(nb * P, (nb + 1) * P)
                    for hch in range(2):
                        hs = slice(hch * 512, (hch + 1) * 512)
                        ps = ps1.tile([P, 512], F32, tag="ps")
                        for d in range(DT):
                            nc.tensor.matmul(
                                ps[:],
                                xb[d][:, ns],
                                wvb[d][:, hs],
                                start=(d == 0),
                                stop=(d == DT - 1),
                            )
                        nc.vector.tensor_add(vt[nb][:, hs], ps[:], bv_sb[:, hs])

            # ---------------- Phase 2: attention ----------------
            with ExitStack() as p2:
                pt_pool = p2.enter_context(tc.tile_pool(name="pt", bufs=1))
                sm = p2.enter_context(tc.tile_pool(name="sm", bufs=4))
                op_pool = p2.enter_context(tc.tile_pool(name="op", bufs=2))
                ps_av = p2.enter_context(tc.tile_pool(name="psav", bufs=4, space="PSUM"))

                pt = [pt_pool.tile([P, 512], BF16, tag=f"pt{j}", name=f"pt{j}") for j in range(NT)]

                for t in range(IT):
                    i0 = 512 * t
                    jmax = 4 * t + 3

                    # scores^T [key j, query i] -> exp -> bf16 p, causal mask
                    # on the diagonal tiles. Columns below the diagonal cut c
                    # are never read by this t's AV matmuls.
                    for j in range(jmax + 1):
                        c = max(0, j * P - i0)
                        w = 512 - c
                        ps = ps_s.tile([P, 512], F32, tag="ps")
                        for hp_ in range(HP):
                            nc.tensor.matmul(
                                ps[:, 0:w],
                                kp[hp_][:, :, j * P:(j + 1) * P],
                                qp[hp_][:, :, i0 + c:i0 + 512],
                                start=(hp_ == 0),
                                stop=(hp_ == HP - 1),
                                perf_mode=DR,
                            )
                        nc.scalar.activation(pt[j][:, c:512], ps[:, 0:w], Exp, scale=float(EXP_SCALE))
                        if c > 0 or j * P == i0:
                            # keep exp where key j*P+p <= query i0+c+f', else 0
                            nc.gpsimd.affine_select(
                                out=pt[j][:, c:512],
                                in_=pt[j][:, c:512],
                                compare_op=mybir.AluOpType.is_ge,
                                fill=0.0,
                                base=0,
                                channel_multiplier=-1,
                                pattern=[[1, w]],
                            )

                    # attn @ V, row-sums, normalize + bias on eviction
                    for s_ in range(4):
                        g = 4 * t + s_
                        qs = slice(s_ * P, (s_ + 1) * P)
                        pav = [ps_av.tile([P, 512], F32, tag="pav", name="pav") for _ in range(2)]
                        prs = ps_rs.tile([P, 1], F32, tag="prs")
                        for j in range(g + 1):
                            lhsT = pt[j][:, qs]
                            for hch in range(2):
                                nc.tensor.matmul(
                                    pav[hch][:],
                                    lhsT,
                                    vt[j][:, hch * 512:(hch + 1) * 512],
                                    start=(j == 0),
                                    stop=(j == g),
                                )
                            nc.tensor.matmul(
                                prs[:], lhsT, ones_bf[:], start=(j == 0), stop=(j == g)
                            )
                        recip = sm.tile([P, 1], F32, tag="recip")
                        nc.vector.reciprocal(recip[:], prs[:])
                        ot = op_pool.tile([P, H], F32, tag="ot")
                        # evict the two H-halves on different engines so they
                        # run in parallel (the second copy otherwise sits on
                        # the tail critical path)
                        nc.scalar.activation(ot[:, 0:512], pav[0][:], Copy, scale=recip[:])
                        nc.vector.tensor_scalar_mul(ot[:, 512:H], pav[1][:], recip[:])
                        rows = slice(i0 + s_ * P, i0 + (s_ + 1) * P)
                        nc.sync.dma_start(out.ap()[rows, 0:512], ot[:, 0:512])
                        nc.sync.dma_start(out.ap()[rows, 512:H], ot[:, 512:H])

    nc.compile()
    return nc


def _get_program():
    key = QK_CHAINS
    if key not in _CACHE:
        _CACHE[key] = build_program(key)
    return _CACHE[key]


def _pair_layout(mat):
    """[D, M] (already fp8) -> [DP, P, 2, M] pair layout, contiguous."""
    d, m_ = mat.shape
    return np.ascontiguousarray(mat.reshape(DP, 2, P, m_).transpose(0, 2, 1, 3))


def prep_inputs(x, Wq, bq, Wk, bk, Wv, bv, qk_chains: int = None):
    if qk_chains is None:
        qk_chains = QK_CHAINS
    x = np.asarray(x, dtype=np.float32)
    Wq = np.asarray(Wq, dtype=np.float32)
    Wk = np.asarray(Wk, dtype=np.float32)
    Wv = np.asarray(Wv, dtype=np.float32)
    bq = np.asarray(bq, dtype=np.float32)
    bv = np.asarray(bv, dtype=np.float32)

    # weight tiles contract over D: rows of W (no transpose) are the pair dim
    wq_l = _pair_layout((Wq * np.float32(WS)).astype(F8NP))
    wk_l = _pair_layout((Wk * np.float32(WS)).astype(F8NP))
    wvb_l = np.ascontiguousarray(Wv.astype(BFNP).reshape(DT, P, H))

    bqS_h = np.ascontiguousarray((bq * np.float32(WS)).reshape(8, P).T)
    bvB_h = np.ascontiguousarray(np.broadcast_to(bv, (P, H))).astype(np.float32)

    in_maps = []
    for b in range(B):
        xb_ = x[b]
        x8 = xb_.astype(F8NP)
        m = {
            "x8d": _pair_layout(np.ascontiguousarray(x8.T)),
            "xbd": np.ascontiguousarray(xb_.T.astype(BFNP).reshape(DT, P, N)),
            "wq8d": wq_l,
            "wk8d": wk_l,
            "wvbd": wvb_l,
            "bqS": bqS_h,
            "bvB": bvB_h,
        }
        if qk_chains >= 2:
            xr8 = (xb_ - x8.astype(np.float32)).astype(F8NP)
            m["xr8d"] = _pair_layout(np.ascontiguousarray(xr8.T))
        in_maps.append(m)
    return in_maps


def kernel(x, Wq, bq, Wk, bk, Wv, bv):
    global LAST_RESULT
    nc = _get_program()
    in_maps = prep_inputs(x, Wq, bq, Wk, bk, Wv, bv)
    res = run_bass_kernel_spmd(nc, in_maps, core_ids=list(range(B)))
    LAST_RESULT = res
    return np.stack([res.results[b]["out"] for b in range(B)], axis=0)
